# revision 7
# baseline (speedup 1.0000x reference)
"""LIF spike (leaky integrate-and-fire with hard reset) Trainium2 kernel.

x: [B=32, T=16, C=128, H=32, W=32] f32  ->  spikes, same shape.
Per element (b,c,h,w), sequential over t:
    v = mem*TAU + x_t ; s = (v >= TH) ; mem = v * (v < TH)

Sharding: batch dim B=32 split across 8 NeuronCores (4 per core), pure
data-parallel SPMD (no collectives).

Per-core layout: for each (b, t) a [C=128 partitions, H*W=1024 free] f32
tile. The membrane potential for each of the 4 local b's stays resident in
SBUF across the T=16 recurrence. Three DVE ops per step:
    scalar_tensor_tensor: v   = (mem * TAU) + x      (fused axpy)
    tensor_scalar:        s   = (v >= TH)            (fp32 2x perf mode)
    scalar_tensor_tensor: mem = (v < TH) * v         (fused hard reset)
DMAs are grouped 4 timesteps at a time (2 MB each) on the HWDGE path.
"""

import sys

import numpy as np

for _p in ("/opt/trn_rl_repo",):
    if _p not in sys.path:
        sys.path.insert(0, _p)

import concourse.bacc as bacc
import concourse.bass as bass
import concourse.mybir as mybir
from concourse.bass_utils import run_bass_kernel_spmd
from concourse.tile import TileContext

B, T, C, H, W = 32, 16, 128, 32, 32
HW = H * W
N_CORES = 8
BL = B // N_CORES  # 4 batches per core
TAU = 0.25
TH = 0.5
TG = 4  # timesteps per DMA group

_nc_cache = None


def _build_nc():
    nc = bacc.Bacc(
        "TRN2", target_bir_lowering=False, debug=False, num_devices=N_CORES
    )
    x = nc.dram_tensor("x", [BL, T, C, HW], mybir.dt.float32, kind="ExternalInput")
    s = nc.dram_tensor("s", [BL, T, C, HW], mybir.dt.float32, kind="ExternalOutput")

    with TileContext(nc) as tc:
        with (
            tc.tile_pool(name="mem", bufs=1) as mp,
            tc.tile_pool(name="xin", bufs=6) as xp,
            tc.tile_pool(name="sout", bufs=6) as sp,
        ):
            mems = []
            for b in range(BL):
                m = mp.tile([C, HW], mybir.dt.float32, tag=f"mem{b}")
                nc.vector.memset(m[:], 0.0)
                mems.append(m)

            for t in range(T):
                for b in range(BL):
                    xt = xp.tile([C, HW], mybir.dt.float32, tag="x")
                    nc.sync.dma_start(out=xt[:], in_=x[b, t])
                    st = sp.tile([C, HW], mybir.dt.float32, tag="s")
                    m = mems[b]
                    v = xt[:]
                    # v = mem*TAU + x_t   (in place over the x tile)
                    nc.vector.scalar_tensor_tensor(
                        out=v,
                        in0=m[:],
                        scalar=TAU,
                        in1=v,
                        op0=mybir.AluOpType.mult,
                        op1=mybir.AluOpType.add,
                    )
                    # s = (v >= TH)
                    nc.vector.tensor_scalar(
                        out=st[:],
                        in0=v,
                        scalar1=TH,
                        scalar2=None,
                        op0=mybir.AluOpType.is_ge,
                    )
                    # mem = (v < TH) * v    (hard reset)
                    nc.vector.scalar_tensor_tensor(
                        out=m[:],
                        in0=v,
                        scalar=TH,
                        in1=v,
                        op0=mybir.AluOpType.is_lt,
                        op1=mybir.AluOpType.mult,
                    )
                    nc.sync.dma_start(out=s[b, t], in_=st[:])
    nc.compile()
    return nc


def _get_nc():
    global _nc_cache
    if _nc_cache is None:
        _nc_cache = _build_nc()
    return _nc_cache


def _ensure_ntff_hook():
    """Install the antenv.axon_hooks shim so trace=True works under axon.

    The agent image's antenv package lacks axon_hooks; build the same
    ctypes-based hook trn_agent_boot would have registered.
    """
    import types

    try:
        from antenv import axon_hooks  # noqa: F401

        return
    except ImportError:
        pass
    import antenv
    from trn_agent_boot.trn_boot import _ntff_profile_via_ctypes

    hook = _ntff_profile_via_ctypes("/opt/axon/libaxon_pjrt.so")
    mod = types.ModuleType("antenv.axon_hooks")
    holder = {"hook": hook}
    mod.set_axon_ntff_profile_hook = lambda h: holder.__setitem__("hook", h)
    mod.get_axon_ntff_profile_hook = lambda: holder["hook"]
    sys.modules["antenv.axon_hooks"] = mod
    antenv.axon_hooks = mod


def kernel(x: np.ndarray, _trace: bool = False, **_unused):
    assert x.shape == (B, T, C, H, W), x.shape
    if _trace:
        _ensure_ntff_hook()
    xr = np.ascontiguousarray(x, dtype=np.float32).reshape(B, T, C, HW)
    nc = _get_nc()
    in_maps = [{"x": xr[i * BL : (i + 1) * BL]} for i in range(N_CORES)]
    res = run_bass_kernel_spmd(
        nc, in_maps, core_ids=list(range(N_CORES)), trace=_trace
    )
    out = np.concatenate([r["s"] for r in res.results], axis=0)
    out = out.reshape(B, T, C, H, W).astype(np.float32, copy=False)
    if _trace:
        kernel.last_results = res
    return out
